# Initial kernel scaffold
#
"""Inverse separable wavelet synthesis (stride-2 transposed conv, 9 taps,
36 -> 12 -> 4 channels, 256x256 -> 512x512) on 8 trn2 NeuronCores.

Formulation: both passes are dense matmuls against host-precomputed banded
operator matrices A_beta [256 in, 512 out] (one per wavelet band), with
symmetric padding + border-mask sign folded into the operators.  H == W so
both passes share the same operators.  The Y pass (contract over h) runs
FIRST because its moving operand is the input in its natural DRAM layout
[h partitions, (w, c) free] - fully contiguous DMA (the X-pass-first variant
needs w on partitions, which forces 144-byte-granule descriptor-bound DMA).

    u[b,n,w,q]   = sum_{by,i} A_by[i,n] * x[b,i,w,9*g2+3*by+bx],  q=(g2,bx)
    out[b,n,m,g2] = sum_{bx,j} A_bx[j,m] * u[b,n,j,3*g2+bx]

Per-core pipeline (pure batch parallelism, 2 images per core):
  stage Y : lhsT = operator window [68 h-in, 128 h2-out] (four overlapping
            input windows, one per output block -> single k-tile each),
            moving = x [h-in part, (g2, w) free] -> PSUM [h2, (g2, w)]
  PE transpose 128x128: u [h2, (q, w)] -> u' [w, (q, h2)]
  stage X : lhsT = operator block [128 w-in, 128 w2-out], moving = u'
            [w-in part, (g2, h2) free] -> PSUM [w2, (g2, h2)]
  PE transpose 128x128: v [w2, (c, h2)] -> osb [h2, (w2, c)] -> DRAM rows
All matmul operands are float32r (fp32 with 11-bit mantissa, full PE rate);
accumulation is exact fp32 in PSUM.  Operator coefficients are dyadic
rationals - exact in f32r; only the input and intermediates get rounded
(~3e-4 relative output error).
"""

import numpy as np
from contextlib import ExitStack

import concourse.bass as bass
import concourse.bacc as bacc
import concourse.mybir as mybir
import concourse.tile as tile
from concourse.bass_utils import run_bass_kernel_spmd

B, H, W, C = 16, 256, 256, 36
NCORES = 8
BPC = B // NCORES  # batches per core
W2 = 2 * W
H2 = 2 * H
F32 = mybir.dt.float32
F32R = mybir.dt.float32r

SMOOTH = [0.0, 0.0, 1.0 / 16.0, 0.5, 14.0 / 16.0, 0.5, 1.0 / 16.0, 0.0, 0.0]
EVEN = [-1.0 / 128.0, -1.0 / 16.0, -10.0 / 64.0, -7.0 / 16.0, 85.0 / 64.0,
        -7.0 / 16.0, -10.0 / 64.0, -1.0 / 16.0, -1.0 / 128.0]
ODD = [1.0 / 256.0, 1.0 / 32.0, 15.0 / 128.0, 17.0 / 32.0, 0.0,
       -17.0 / 32.0, -15.0 / 128.0, -1.0 / 32.0, -1.0 / 256.0]

# Stage X: which 128-row k-tiles of u' feed each 128-col w2 output block
# (out block n covers in rows [64n-2, 64n+65]).
KTS = {0: (0,), 1: (0, 1), 2: (0, 1), 3: (1,)}
# Stage Y input windows (one 68-row window per 128-row h2 output block).
W0 = [0, 62, 126, 188]
KW = 68


def _build_operator_full():
    """[3 bands, 256 in-rows, 512 out-cols] float64 folded operator."""
    inv = np.array([SMOOTH, EVEN, ODD], dtype=np.float64)
    S = 256
    Sp = S + 6
    j = np.arange(Sp)[:, None]
    m = np.arange(2 * S)[None, :]
    t = m + 10 - 2 * j
    valid = (t >= 0) & (t <= 8)
    P = np.zeros((3, Sp, 2 * S))
    for b in range(3):
        P[b][valid] = inv[b][t[valid]]
    # border mask: odd band negated on the 3-wide padded border
    P[2, [0, 1, 2, Sp - 3, Sp - 2, Sp - 1], :] *= -1.0
    # fold symmetric padding: pad[0..2]=x[2],x[1],x[0]; pad[-3:]=x[-1],x[-2],x[-3]
    A = P[:, 3:3 + S].copy()
    A[:, 2] += P[:, 0]
    A[:, 1] += P[:, 1]
    A[:, 0] += P[:, 2]
    A[:, S - 1] += P[:, Sp - 3]
    A[:, S - 2] += P[:, Sp - 2]
    A[:, S - 3] += P[:, Sp - 1]
    return A


def _build_operator_array():
    """Stage-X operator: [3 bands, 2 ktiles, 128 in-rows, 512 out-cols] f32."""
    A = _build_operator_full()
    return np.ascontiguousarray(A.reshape(3, 2, 128, 512).astype(np.float32))


def _build_operator_windows():
    """Stage-Y operator: [3 bands, 4 blocks, 68 in-rows, 128 out-cols] f32."""
    A = _build_operator_full()
    out = np.zeros((3, 4, KW, 128), np.float64)
    for blk in range(4):
        out[:, blk] = A[:, W0[blk]:W0[blk] + KW, blk * 128:(blk + 1) * 128]
    return np.ascontiguousarray(out.astype(np.float32))


def _build_program(repeat=1):
    nc = bacc.Bacc("TRN2", target_bir_lowering=False)
    x = nc.declare_dram_parameter("x", [BPC, H, W, C], F32R, isOutput=False)
    a_w = nc.declare_dram_parameter("a_w", [3, 4, KW, 128], F32R, isOutput=False)
    a_op = nc.declare_dram_parameter("a_op", [3, 2, 128, W2], F32R, isOutput=False)
    ident = nc.declare_dram_parameter("ident", [128, 128], F32R, isOutput=False)
    out = nc.declare_dram_parameter("out", [BPC, H2, W2, 4], F32, isOutput=True)

    with tile.TileContext(nc) as tc, ExitStack() as ctx:
        const = ctx.enter_context(tc.tile_pool(name="const", bufs=1))
        xpool = ctx.enter_context(tc.tile_pool(name="xp", bufs=4))
        upool = ctx.enter_context(tc.tile_pool(name="up", bufs=2))
        vpool = ctx.enter_context(tc.tile_pool(name="vp", bufs=1))
        wpool = ctx.enter_context(tc.tile_pool(name="wp", bufs=3))
        opool = ctx.enter_context(tc.tile_pool(name="op", bufs=2))
        psY = ctx.enter_context(tc.tile_pool(name="psY", bufs=3, space="PSUM"))
        psT = ctx.enter_context(tc.tile_pool(name="psT", bufs=2, space="PSUM"))
        psX = ctx.enter_context(tc.tile_pool(name="psX", bufs=2, space="PSUM"))

        aw_sb = {}
        for beta in range(3):
            for blk in range(4):
                t = const.tile([KW, 128], F32R, name=f"aw_{beta}_{blk}",
                               tag=f"aw_{beta}_{blk}")
                nc.sync.dma_start(t[:], a_w[beta, blk])
                aw_sb[beta, blk] = t
        a_sb = {}
        for beta in range(3):
            for kt in range(2):
                t = const.tile([128, W2], F32R, name=f"a_{beta}_{kt}",
                               tag=f"a_{beta}_{kt}")
                nc.sync.dma_start(t[:], a_op[beta, kt])
                a_sb[beta, kt] = t
        ident_sb = const.tile([128, 128], F32R, name="ident_sb", tag="ident")
        nc.sync.dma_start(ident_sb[:], ident[:])

        for rep in range(repeat):
          for b in range(BPC):
            rb = rep * BPC + b
            # u[h2blk]: [128 h2, (q=12 ch, w=256)], q = 3*g2 + bx
            u = {}
            for blk in range(4):
                u[blk] = upool.tile([128, 12 * W], F32R,
                                    name=f"u_{rb}_{blk}", tag=f"u_{blk % 2}")
            # ---- stage Y: contract h (natural-layout loads) ----
            for wc in range(2):
                xw = {}
                for win in range(4):
                    xt = xpool.tile([KW, 128 * C], F32R,
                                    name=f"x_{rb}_{wc}_{win}", tag="x")
                    src = x[b, W0[win]:W0[win] + KW,
                            wc * 128:(wc + 1) * 128, :]
                    nc.sync.dma_start(
                        xt.rearrange("h (w c) -> h w c", c=C), src)
                    xw[win] = xt
                for blk in range(4):
                    uv = u[blk].rearrange("p (q w) -> p q w", q=12)
                    for bx in range(3):
                        ps = psY.tile([128, 512], F32,
                                      name=f"psY_{rb}_{wc}_{blk}_{bx}",
                                      tag="psY")
                        psv = ps.rearrange("p (g w) -> p g w", g=4)
                        for i, by in enumerate(range(3)):
                            # channels c = 9*g2 + 3*by + bx
                            rhs = xw[blk].rearrange(
                                "h (w g2 e c) -> h g2 e c w",
                                g2=4, e=3, c=3)[:, :, by, bx, :]
                            nc.tensor.matmul(psv, aw_sb[by, blk][:], rhs,
                                             start=(i == 0), stop=(i == 2))
                        # scatter into u: q = 3*g2 + bx
                        dst = u[blk].rearrange(
                            "p (g2 e w) -> p e g2 w", g2=4, e=3)[
                                :, bx, :, wc * 128:(wc + 1) * 128]
                        nc.vector.tensor_copy(out=dst, in_=psv)
            # ---- mid transposes + stage X, streamed per h2 block ----
            # v[w2blk]: [128 w2, (c=4, h2=512)]
            v = {}
            for blk in range(4):
                v[blk] = vpool.tile([128, 4 * H2], F32R,
                                    name=f"v_{rb}_{blk}", tag=f"v_{blk}")
            for h2b in range(4):
                # transpose u[h2b] [h2, (q, w)] -> up[wt] [w, (q, h2-slice)]
                up = {}
                for wt in range(2):
                    up[wt] = wpool.tile([128, 12 * 128], F32R,
                                        name=f"up_{rb}_{h2b}_{wt}", tag="up")
                uvb = u[h2b].rearrange("p (q w) -> p q w", q=12)
                for wt in range(2):
                    for q4 in range(3):
                        pt = psT.tile([128, 512], F32R,
                                      name=f"psT_{rb}_{h2b}_{wt}_{q4}",
                                      tag="psT")
                        for i in range(4):
                            q = q4 * 4 + i
                            nc.tensor.transpose(
                                pt[:, i * 128:(i + 1) * 128],
                                uvb[:, q, wt * 128:(wt + 1) * 128],
                                ident_sb[:])
                        dst = up[wt].rearrange("p (q h) -> p q h", q=12)[
                            :, q4 * 4:(q4 + 1) * 4, :]
                        src = pt.rearrange("p (q h) -> p q h", q=4)
                        if (wt + q4) % 2 == 0:
                            nc.vector.tensor_copy(out=dst, in_=src)
                        else:
                            nc.scalar.copy(out=dst, in_=src)
                # stage X for this h2 slice
                for w2b in range(4):
                    ps = psX.tile([128, 512], F32,
                                  name=f"psX_{rb}_{h2b}_{w2b}", tag="psX")
                    psv = ps.rearrange("p (g h) -> p g h", g=4)
                    mms = [(bx, kt) for bx in range(3) for kt in KTS[w2b]]
                    for i, (bx, kt) in enumerate(mms):
                        lhsT = a_sb[bx, kt][:, w2b * 128:(w2b + 1) * 128]
                        # q = 3*g2 + bx -> fixed bx, g2 strided by 3
                        rhs = up[kt].rearrange(
                            "p (g2 e h) -> p e g2 h", g2=4, e=3)[:, bx, :, :]
                        nc.tensor.matmul(psv, lhsT, rhs,
                                         start=(i == 0),
                                         stop=(i == len(mms) - 1))
                    dst = v[w2b].rearrange("p (c h) -> p c h", c=4)[
                        :, :, h2b * 128:(h2b + 1) * 128]
                    if (h2b + w2b) % 2 == 0:
                        nc.scalar.copy(out=dst, in_=psv)
                    else:
                        nc.vector.tensor_copy(out=dst, in_=psv)
            # ---- output transposes: v [w2, (c, h2)] -> osb [h2, (w2, c)] ----
            for h2t in range(4):
                osb = opool.tile([128, W2 * 4], F32, name=f"osb_{rb}_{h2t}",
                                 tag="osb")
                osbv = osb.rearrange("p (w c) -> p c w", c=4)
                for w2b in range(4):
                    pt = psT.tile([128, 512], F32,
                                  name=f"psO_{rb}_{h2t}_{w2b}", tag="psT")
                    vv = v[w2b].rearrange("p (c h) -> p c h", c=4)
                    for c in range(4):
                        nc.tensor.transpose(
                            pt[:, c * 128:(c + 1) * 128].bitcast(F32R),
                            vv[:, c, h2t * 128:(h2t + 1) * 128],
                            ident_sb[:])
                    dst = osbv[:, :, w2b * 128:(w2b + 1) * 128]
                    src = pt.rearrange("p (c w) -> p c w", c=4)
                    if w2b % 2 == 0:
                        nc.vector.tensor_copy(out=dst, in_=src)
                    else:
                        nc.scalar.copy(out=dst, in_=src)
                dstd = out[b, h2t * 128:(h2t + 1) * 128, :, :].rearrange(
                    "h w c -> h (w c)")
                nc.sync.dma_start(dstd, osb[:])
    nc.compile()
    return nc


def _round_fp32r(x):
    """Round fp32 array to fp32r (fp32 with 11-bit mantissa, RNE) on host."""
    b = x.view(np.uint32).astype(np.uint64)
    b = (b + 0x7FF + ((b >> 12) & 1)) & ~np.uint64(0xFFF)
    return b.astype(np.uint32).view(np.float32)


_PROGRAMS = {}


def _get_program(repeat=1, mode=None, phases=None):
    if repeat not in _PROGRAMS:
        _PROGRAMS[repeat] = _build_program(repeat)
    return _PROGRAMS[repeat]


def _host_inputs(inputs):
    a4 = _build_operator_array()
    aw = _build_operator_windows()
    identity = np.ascontiguousarray(np.eye(128, dtype=np.float32))
    shards = _round_fp32r(inputs).reshape(NCORES, BPC, H, W, C)
    return [{"x": np.ascontiguousarray(shards[c]), "a_op": a4, "a_w": aw,
             "ident": identity} for c in range(NCORES)]


def _run(inputs, trace=False, tmpdir=None, repeat=1, mode=None):
    """Returns (full output [16,512,512,4], BassKernelResults)."""
    inputs = np.ascontiguousarray(np.asarray(inputs, dtype=np.float32))
    assert inputs.shape == (B, H, W, C), inputs.shape
    nc = _get_program(repeat)
    in_maps = _host_inputs(inputs)
    res = run_bass_kernel_spmd(nc, in_maps, core_ids=list(range(NCORES)),
                               trace=trace, tmpdir=tmpdir)
    outs = [np.asarray(res.results[c]["out"]) for c in range(NCORES)]
    full = np.concatenate(outs, axis=0).astype(np.float32)
    return full, res


def kernel(inputs):
    full, _ = _run(inputs)
    return full



# revision 2
# speedup vs baseline: 1.0190x; 1.0190x over previous
"""Inverse separable wavelet synthesis (stride-2 transposed conv, 9 taps,
36 -> 12 -> 4 channels, 256x256 -> 512x512) on 8 trn2 NeuronCores.

Formulation: both passes are dense matmuls against host-precomputed banded
operator matrices A_beta [256 in, 512 out] (one per wavelet band), with
symmetric padding + border-mask sign folded into the operators.  All filter
coefficients are dyadic rationals with <= 8 mantissa bits -> EXACT in bf16,
so everything on-chip runs in bf16 (input and intermediates round to bf16;
PSUM accumulation stays exact fp32).

Host side: input is permuted to [b, h, by, g2, bx, w] (band 'by' outermost
of the channel split c = 9*g2 + 3*by + bx) and cast to bf16.  This makes
every matmul moving-operand access pattern contiguous in 256-byte runs
(full 16B-SBUF-cacheline hits) and halves input DMA bytes.

Input DMA descriptors are split to 4608 B (max_dma_last_dim) so each load
instruction carries 272 descriptors -> the HWDGE spreads them over all 16
SDMA engines (68 x 18KB descriptors land on only 4 engines = the previous
bottleneck).

Per-core pipeline (pure batch parallelism, 2 images per core), fully
streamed per 128-row h2-block:
  load x window [68 h, (by g2 bx w)]  (one DMA, 272 descriptors)
  stage Y : lhsT = A_by window [68, 128 h2], moving = x win [68, (g2, w)]
            -> psY [h2, (g2, w-half)] (3-band accumulation), cast -> u bf16
  PE transpose u [h2, (q, w)] -> up [w, (q, h2)]      (q = 3*g2 + bx)
  stage X : lhsT = A_bx block [128 w, 128 w2], moving = up [w, (g2, h2)]
            -> psX [w2, (g2, h2)], cast -> v bf16
  PE transpose v [w2, (c, h2)] -> osb [h2, (w2, c)] fp32 -> DRAM rows
"""

import numpy as np
import ml_dtypes
from contextlib import ExitStack

import concourse.bass as bass
import concourse.bacc as bacc
import concourse.mybir as mybir
import concourse.tile as tile
from concourse.bass_utils import run_bass_kernel_spmd

B, H, W, C = 16, 256, 256, 36
NCORES = 8
BPC = B // NCORES  # batches per core
W2 = 2 * W
H2 = 2 * H
F32 = mybir.dt.float32
BF16 = mybir.dt.bfloat16

SMOOTH = [0.0, 0.0, 1.0 / 16.0, 0.5, 14.0 / 16.0, 0.5, 1.0 / 16.0, 0.0, 0.0]
EVEN = [-1.0 / 128.0, -1.0 / 16.0, -10.0 / 64.0, -7.0 / 16.0, 85.0 / 64.0,
        -7.0 / 16.0, -10.0 / 64.0, -1.0 / 16.0, -1.0 / 128.0]
ODD = [1.0 / 256.0, 1.0 / 32.0, 15.0 / 128.0, 17.0 / 32.0, 0.0,
       -17.0 / 32.0, -15.0 / 128.0, -1.0 / 32.0, -1.0 / 256.0]

# Stage X: which 128-row k-tiles of up feed each 128-col w2 output block
# (out block n covers in rows [64n-2, 64n+65]).
KTS = {0: (0,), 1: (0, 1), 2: (0, 1), 3: (1,)}
# Stage Y input windows (one 68-row window per 128-row h2 output block).
W0 = [0, 62, 126, 188]
KW = 68


def _build_operator_full():
    """[3 bands, 256 in-rows, 512 out-cols] float64 folded operator."""
    inv = np.array([SMOOTH, EVEN, ODD], dtype=np.float64)
    S = 256
    Sp = S + 6
    j = np.arange(Sp)[:, None]
    m = np.arange(2 * S)[None, :]
    t = m + 10 - 2 * j
    valid = (t >= 0) & (t <= 8)
    P = np.zeros((3, Sp, 2 * S))
    for b in range(3):
        P[b][valid] = inv[b][t[valid]]
    # border mask: odd band negated on the 3-wide padded border
    P[2, [0, 1, 2, Sp - 3, Sp - 2, Sp - 1], :] *= -1.0
    # fold symmetric padding: pad[0..2]=x[2],x[1],x[0]; pad[-3:]=x[-1],x[-2],x[-3]
    A = P[:, 3:3 + S].copy()
    A[:, 2] += P[:, 0]
    A[:, 1] += P[:, 1]
    A[:, 0] += P[:, 2]
    A[:, S - 1] += P[:, Sp - 3]
    A[:, S - 2] += P[:, Sp - 2]
    A[:, S - 3] += P[:, Sp - 1]
    return A


def _build_operator_array():
    """Stage-X operator: [3 bands, 2 ktiles, 128 in-rows, 512 out-cols] bf16."""
    A = _build_operator_full()
    return np.ascontiguousarray(
        A.reshape(3, 2, 128, 512).astype(ml_dtypes.bfloat16))


def _build_operator_windows():
    """Stage-Y operator: [3 bands, 4 blocks, 68 in-rows, 128 out-cols] bf16."""
    A = _build_operator_full()
    out = np.zeros((3, 4, KW, 128), np.float64)
    for blk in range(4):
        out[:, blk] = A[:, W0[blk]:W0[blk] + KW, blk * 128:(blk + 1) * 128]
    return np.ascontiguousarray(out.astype(ml_dtypes.bfloat16))


def _build_program(repeat=1):
    nc = bacc.Bacc("TRN2", target_bir_lowering=False)
    # x: [b, h, by, g2, bx, w] bf16 (host-permuted, band-outermost channels)
    x = nc.declare_dram_parameter("x", [BPC, H, 3, 4, 3, W], BF16,
                                  isOutput=False)
    a_w = nc.declare_dram_parameter("a_w", [3, 4, KW, 128], BF16,
                                    isOutput=False)
    a_op = nc.declare_dram_parameter("a_op", [3, 2, 128, W2], BF16,
                                     isOutput=False)
    ident = nc.declare_dram_parameter("ident", [128, 128], BF16,
                                      isOutput=False)
    out = nc.declare_dram_parameter("out", [BPC, H2, W2, 4], F32,
                                    isOutput=True)

    with tile.TileContext(nc) as tc, ExitStack() as ctx:
        const = ctx.enter_context(tc.tile_pool(name="const", bufs=1))
        xpool = ctx.enter_context(tc.tile_pool(name="xp", bufs=3))
        upool = ctx.enter_context(tc.tile_pool(name="up", bufs=2))
        wpool = ctx.enter_context(tc.tile_pool(name="wp", bufs=2))
        vpool = ctx.enter_context(tc.tile_pool(name="vp", bufs=2))
        opool = ctx.enter_context(tc.tile_pool(name="op", bufs=3))
        psY = ctx.enter_context(tc.tile_pool(name="psY", bufs=2, space="PSUM"))
        psT = ctx.enter_context(tc.tile_pool(name="psT", bufs=2, space="PSUM"))
        psX = ctx.enter_context(tc.tile_pool(name="psX", bufs=2, space="PSUM"))
        psO = ctx.enter_context(tc.tile_pool(name="psO", bufs=2, space="PSUM"))

        aw_sb = {}
        for beta in range(3):
            for blk in range(4):
                t = const.tile([KW, 128], BF16, name=f"aw_{beta}_{blk}",
                               tag=f"aw_{beta}_{blk}")
                nc.sync.dma_start(t[:], a_w[beta, blk])
                aw_sb[beta, blk] = t
        a_sb = {}
        for beta in range(3):
            for kt in range(2):
                t = const.tile([128, W2], BF16, name=f"a_{beta}_{kt}",
                               tag=f"a_{beta}_{kt}")
                nc.sync.dma_start(t[:], a_op[beta, kt])
                a_sb[beta, kt] = t
        ident_sb = const.tile([128, 128], BF16, name="ident_sb", tag="ident")
        nc.sync.dma_start(ident_sb[:], ident[:])

        for rep in range(repeat):
          for b in range(BPC):
            rb = rep * BPC + b
            for blk in range(4):
                # ---- load x window [68, (by, g2, bx, w)], 272 descriptors
                xt = xpool.tile([KW, 3 * 12 * W], BF16,
                                name=f"x_{rb}_{blk}", tag="x")
                src = x[b, W0[blk]:W0[blk] + KW].rearrange(
                    "h by g2 bx w -> h (by g2 bx w)")
                eng = nc.sync if blk % 2 == 0 else nc.scalar
                eng.dma_start(xt[:], src, max_dma_last_dim=4608)
                xv = xt.rearrange("h (by g2 bx w) -> h by g2 bx w",
                                  by=3, g2=4, bx=3)

                # ---- stage Y: u[blk] [h2 128, (g2, bx, w)] bf16
                u = upool.tile([128, 12 * W], BF16, name=f"u_{rb}_{blk}",
                               tag="u")
                uv = u.rearrange("p (g2 bx w) -> p g2 bx w", g2=4, bx=3)
                for bx in range(3):
                    for wc in range(2):
                        ps = psY.tile([128, 512], F32,
                                      name=f"psY_{rb}_{blk}_{bx}_{wc}",
                                      tag="psY")
                        psv = ps.rearrange("p (g w) -> p g w", g=4)
                        for i, by in enumerate(range(3)):
                            rhs = xv[:, by, :, bx, wc * 128:(wc + 1) * 128]
                            nc.tensor.matmul(psv, aw_sb[by, blk][:], rhs,
                                             start=(i == 0), stop=(i == 2))
                        dst = uv[:, :, bx, wc * 128:(wc + 1) * 128]
                        if (bx + wc) % 2 == 0:
                            nc.vector.tensor_copy(out=dst, in_=psv)
                        else:
                            nc.scalar.copy(out=dst, in_=psv)

                # ---- mid transpose: u [h2, (q, w)] -> up[wt] [w, (q, h2)]
                up = {}
                for wt in range(2):
                    up[wt] = wpool.tile([128, 12 * 128], BF16,
                                        name=f"up_{rb}_{blk}_{wt}", tag="upt")
                uvb = u.rearrange("p (q w) -> p q w", q=12)
                for wt in range(2):
                    for q4 in range(3):
                        pt = psT.tile([128, 512], BF16,
                                      name=f"psT_{rb}_{blk}_{wt}_{q4}",
                                      tag="psT")
                        for i in range(4):
                            q = q4 * 4 + i
                            nc.tensor.transpose(
                                pt[:, i * 128:(i + 1) * 128],
                                uvb[:, q, wt * 128:(wt + 1) * 128],
                                ident_sb[:])
                        dst = up[wt].rearrange("p (q h) -> p q h", q=12)[
                            :, q4 * 4:(q4 + 1) * 4, :]
                        src2 = pt.rearrange("p (q h) -> p q h", q=4)
                        if (wt + q4) % 2 == 0:
                            nc.vector.tensor_copy(out=dst, in_=src2)
                        else:
                            nc.scalar.copy(out=dst, in_=src2)

                # ---- stage X for this h2 block -> v[w2b] [w2, (c, h2slice)]
                v = {}
                for w2b in range(4):
                    ps = psX.tile([128, 512], F32,
                                  name=f"psX_{rb}_{blk}_{w2b}", tag="psX")
                    psv = ps.rearrange("p (g h) -> p g h", g=4)
                    mms = [(bx, kt) for bx in range(3) for kt in KTS[w2b]]
                    for i, (bx, kt) in enumerate(mms):
                        lhsT = a_sb[bx, kt][:, w2b * 128:(w2b + 1) * 128]
                        # up free order q = 3*g2 + bx -> fix bx, stride g2
                        rhs = up[kt].rearrange(
                            "p (g2 e h) -> p e g2 h", g2=4, e=3)[:, bx, :, :]
                        nc.tensor.matmul(psv, lhsT, rhs,
                                         start=(i == 0),
                                         stop=(i == len(mms) - 1))
                    vt = vpool.tile([128, 4 * 128], BF16,
                                    name=f"v_{rb}_{blk}_{w2b}",
                                    tag=f"v_{w2b}")
                    dst = vt.rearrange("p (c h) -> p c h", c=4)
                    if w2b % 2 == 0:
                        nc.scalar.copy(out=dst, in_=psv)
                    else:
                        nc.vector.tensor_copy(out=dst, in_=psv)
                    v[w2b] = vt

                # ---- output transpose: v [w2, (c, h2)] -> osb [h2, (w2, c)]
                osb = opool.tile([128, W2 * 4], F32, name=f"osb_{rb}_{blk}",
                                 tag="osb")
                osbv = osb.rearrange("p (w c) -> p c w", c=4)
                for w2b in range(4):
                    pt = psO.tile([128, 512], BF16,
                                  name=f"psO_{rb}_{blk}_{w2b}", tag="psO")
                    vv = v[w2b].rearrange("p (c h) -> p c h", c=4)
                    for c in range(4):
                        nc.tensor.transpose(
                            pt[:, c * 128:(c + 1) * 128],
                            vv[:, c, :],
                            ident_sb[:])
                    dst = osbv[:, :, w2b * 128:(w2b + 1) * 128]
                    src2 = pt.rearrange("p (c w) -> p c w", c=4)
                    if w2b % 2 == 0:
                        nc.vector.tensor_copy(out=dst, in_=src2)
                    else:
                        nc.scalar.copy(out=dst, in_=src2)
                dstd = out[b, blk * 128:(blk + 1) * 128, :, :].rearrange(
                    "h w c -> h (w c)")
                nc.sync.dma_start(dstd, osb[:])
    nc.compile()
    return nc


_PROGRAMS = {}


def _get_program(repeat=1):
    if repeat not in _PROGRAMS:
        _PROGRAMS[repeat] = _build_program(repeat)
    return _PROGRAMS[repeat]


def _host_inputs(inputs):
    a4 = _build_operator_array()
    aw = _build_operator_windows()
    identity = np.ascontiguousarray(np.eye(128, dtype=ml_dtypes.bfloat16))
    # [B,H,W,C] c = 9*g2 + 3*by + bx -> [B, H, by, g2, bx, W] bf16
    xp = inputs.reshape(B, H, W, 4, 3, 3).transpose(0, 1, 4, 3, 5, 2)
    xp = np.ascontiguousarray(xp.astype(ml_dtypes.bfloat16))
    shards = xp.reshape(NCORES, BPC, H, 3, 4, 3, W)
    return [{"x": np.ascontiguousarray(shards[c]), "a_op": a4, "a_w": aw,
             "ident": identity} for c in range(NCORES)]


def _run(inputs, trace=False, tmpdir=None, repeat=1):
    """Returns (full output [16,512,512,4], BassKernelResults)."""
    inputs = np.ascontiguousarray(np.asarray(inputs, dtype=np.float32))
    assert inputs.shape == (B, H, W, C), inputs.shape
    nc = _get_program(repeat)
    in_maps = _host_inputs(inputs)
    res = run_bass_kernel_spmd(nc, in_maps, core_ids=list(range(NCORES)),
                               trace=trace, tmpdir=tmpdir)
    outs = [np.asarray(res.results[c]["out"]) for c in range(NCORES)]
    full = np.concatenate(outs, axis=0).astype(np.float32)
    return full, res


def kernel(inputs):
    full, _ = _run(inputs)
    return full
